# revision 23
# baseline (speedup 1.0000x reference)
"""LIF spiking-neuron kernel for Trainium2 (Bass/Tile), 8-core SPMD.

Problem: x [B=32, T=8, C=128, H=32, W=32] f32.  Per (b,c,h,w) neuron,
sequential over T:
    mem = mem*TAU + x_t;  spike = (mem - 1 > 0);  mem = 0 if spike
TAU = 0.5, THRESH = 1.0.

Sharding: batch dim B=32 split 4-per-core across 8 NeuronCores; the
recurrence is along T only, so there is no communication.

Per-core algorithm (bit-exact vs the fp32 reference):
  TAU = 0.5 is a power of two, so rescale the state M_t = 2^t * m_t.
  The decay becomes a pure add:  M_t = M_{t-1} + 2^t * x_t  (the 2^t
  prescale of x is exact in fp32, and power-of-2 scaling commutes with
  fp rounding, so every M_t is bit-exactly 2^t * m_t).
  spike_t = (M_t > 2^t)  <=>  (m_t > 1)  <=>  reference's (m_t - 1 > 0).

  The state update (previous step's reset + prescale + accumulate) is
  ONE fused custom-DVE op per step:
      M_t = select(M_{t-1} > 2^(t-1), 0, M_{t-1}) + x_t * 2^t
  so spike extraction is an output tap off the recurrence critical path.

Output compression (the kernel is HBM-bandwidth bound, so output bytes
are wall-clock):  ACT's Sign gives the trit sigma_t = sign(M_t - 2^t)
in {-1,0,+1} exactly, in fp8 -- spike <=> sigma=+1.  For t=0..3 the
idle PE packs trits into one u8 plane with diagonal-matrix matmuls
accumulating in PSUM:
    plane_lo = 85 + sum_{t=0..3} 4^t sigma_t   in [0, 170]
(balanced-quaternary digits; the +85 bias is applied by the PSUM->u8
cast).  All of that completes mid-stream.  t=4..6 are raw u8 spikes via
Sign -> u8 (the saturating cast maps -1 to 0, verified on HW); t=7 is a
fused LIF_SPIKE custom op (no membrane materialized).  Output: 5 u8
planes [B,C,H,W] = 2.6 MB/core vs 16.8 MB fp32.  Host decodes.

Engine split:
  DVE:  12 LIF_STEP + 2 LIF_SPIKE (the only recurrence work)
  ACT:  Sign trits/spikes + one biased PSUM->u8 cast per chain
  PE :  trit-packing matmuls (4^t diagonal fp8 weights, PSUM accumulate)
  SP :  single in-order DMA queue: inputs then outputs (so output
        traffic never delays the input stream that paces the chain)
  GPS:  nothing -- its software ops are ~15x slower than DVE and it
        shares SBUF ports with DVE (running anything there starves the
        recurrence).
"""

import re

import numpy as np

from concourse import bacc, bass, mybir, tile
from concourse import dve_ops
from concourse.bass_utils import run_bass_kernel_spmd
from concourse.dve_spec import Spec, Src0, Src1, C0, C1, Zero, select

# Full-problem shape (hardcoded per harness contract).
B, T, C, H, W = 32, 8, 128, 32, 32
N_CORES = 8
B_LOC = B // N_CORES          # 4 batches per core
F = H * W                     # 1024 free elements per (b, t, c)
FP32 = mybir.dt.float32
U8 = mybir.dt.uint8
FP8 = mybir.dt.float8e4

PAIR = 2                      # batches fused per tile
G = B_LOC // PAIR             # chain groups per core
FW = PAIR * F                 # 2048 free elements per tile
BANK = 512                    # PSUM bank: one matmul output's max f32 cols

N_LO = 4                      # t=0..3 trit-packed into plane_lo
BIAS_LO = 85.0                # (4^4 - 1) / 3

_NC_CACHE = {}


def _register(name, body, ref):
    """Register a custom DVE op (idempotent), pinning uops_sha in-process."""
    if name in dve_ops._SUB_OPCODE_FOR_NAME:
        return next(op for op in dve_ops.OPS if op.name == name)
    op = dve_ops.DveOp(name, Spec(body=body, reference=ref), subdim=False, uops_sha={})
    dve_ops.OPS.append(op)
    dve_ops.CUSTOM_DVE_SPECS[name] = op.spec
    dve_ops._SUB_OPCODE_FOR_NAME[name] = (
        dve_ops._CUSTOM_DVE_ROW_BASE + len(dve_ops.OPS) - 1
    )
    for ver in ("v3", "v4"):
        try:
            op.compile(ver)
        except ValueError as e:
            m = re.search(r'"%s"\]="([0-9a-f]{16})"' % ver, str(e))
            if not m:
                raise
            op.uops_sha[ver] = m.group(1)
            dve_ops._COMPILE_CACHE.pop((name, ver), None)
            op.compile(ver)
    return op


# M_t = select(M_{t-1} > 2^(t-1), 0, M_{t-1}) + x_t * 2^t
LIF_STEP = _register(
    "LIF_STEP_ANT",
    select(Src1 > C1, Zero, Src1) + Src0 * C0,
    lambda in0, in1, s0, s1, imm2: (
        np.where(in1 > s1, np.float32(0.0), in1) + in0 * s0
    ).astype(np.float32),
)
# spike_t = (select(M_{t-1} > 2^(t-1), 0, M_{t-1}) + x_t * 2^t) > 2^t
LIF_SPIKE = _register(
    "LIF_SPIKE_ANT",
    (select(Src1 > C1, Zero, Src1) + Src0 * C0) > C0,
    lambda in0, in1, s0, s1, imm2: (
        (np.where(in1 > s1, np.float32(0.0), in1) + in0 * s0) > s0
    ).astype(np.float32),
)


def _emit(tc, x_d, w_d, o_lo, o_raw):
    nc = tc.nc

    def dram_x(g, t):
        return x_d[g * PAIR : (g + 1) * PAIR, t].rearrange("p c h w -> c p (h w)")

    def dram_o(ap, g):
        return ap[g * PAIR : (g + 1) * PAIR].rearrange("p c h w -> c p (h w)")

    def as3(tile_ap):
        return tile_ap.rearrange("c (p f) -> c p f", p=PAIR)

    with (
        tc.tile_pool(name="xp", bufs=12) as xp,
        tc.tile_pool(name="sg", bufs=4) as sgp,
        tc.tile_pool(name="op", bufs=10) as op_,
        tc.tile_pool(name="mp", bufs=6) as mp,
        tc.tile_pool(name="bp", bufs=1) as bp,
        tc.tile_pool(name="ps", bufs=G, space=bass.MemorySpace.PSUM) as ps,
    ):
        # constants: -2^t Sign biases (t<=6) and the cast bias
        biases = []
        for t in range(T - 1):
            bt = bp.tile([C, 1], FP32, name=f"bias{t}")
            nc.vector.memset(bt, -float(2.0**t))
            biases.append(bt)
        bias_lo = bp.tile([C, 1], FP32, name="bias_lo")
        nc.vector.memset(bias_lo, BIAS_LO)

        # 4^t diagonal fp8 pack weights: one small contiguous DMA on the
        # ACT queue so the SP input stream starts clean
        wall = bp.tile([C, N_LO * C], FP8, name="wall")
        nc.scalar.dma_start(out=wall, in_=w_d)
        wts = [wall[:, j * C : (j + 1) * C] for j in range(N_LO)]

        # --- all input DMAs up front on the single in-order SP queue;
        # t=0 lands directly in the chain's first membrane tile.
        with tc.high_priority():
            ms = {}
            for g in range(G):
                m0 = mp.tile([C, FW], FP32, name="mt")
                nc.sync.dma_start(out=as3(m0), in_=dram_x(g, 0))
                ms[g] = m0
            xs = {}
            for t in range(1, T):
                for g in range(G):
                    xt = xp.tile([C, FW], FP32)
                    nc.sync.dma_start(out=as3(xt), in_=dram_x(g, t))
                    xs[(t, g)] = xt

        accs = [ps.tile([C, FW], FP32, name="acc") for _ in range(G)]

        # Output DMAs carry a virtual-time floor so the scheduler orders
        # them AFTER every input DMA in the in-order SP queue: output
        # descriptors must not share DMA-engine bandwidth with the late
        # input tiles that pace the recurrence tail.
        out_seq = [0]

        def out_dma(o_ap, g, tile_ap):
            with tc.tile_wait_until(0.055 + 0.001 * out_seq[0]):
                nc.sync.dma_start(out=dram_o(o_ap, g), in_=as3(tile_ap))
            out_seq[0] += 1

        # --- recurrence (DVE) + spike taps + packing
        for t in range(T):
            th = float(2.0**t)
            for g in range(G):
                if 0 < t < T - 1:
                    m_new = mp.tile([C, FW], FP32, name="mt")
                    nc.vector._custom_dve(
                        LIF_STEP, out=m_new, in0=xs[(t, g)], in1=ms[g],
                        s0=th, s1=th / 2.0,
                    )
                    ms[g] = m_new
                if t < N_LO:
                    # trit sigma_t = Sign(M - 2^t) in fp8, packed by PE
                    sg = sgp.tile([C, FW], FP8, name="sgt")
                    nc.scalar.activation(
                        sg, ms[g], mybir.ActivationFunctionType.Sign, bias=biases[t]
                    )
                    for j in range(FW // BANK):
                        sl = slice(j * BANK, (j + 1) * BANK)
                        nc.tensor.matmul(
                            accs[g][:, sl], wts[t], sg[:, sl],
                            start=(t == 0), stop=(t == N_LO - 1),
                        )
                    if t == N_LO - 1:
                        pk = op_.tile([C, FW], U8, name="pk")
                        nc.scalar.activation(
                            pk, accs[g],
                            mybir.ActivationFunctionType.Identity, bias=bias_lo,
                        )
                        out_dma(o_lo, g, pk)
                elif t < T - 1:
                    # raw u8 spike: Sign's u8 cast saturates -1 -> 0
                    s = op_.tile([C, FW], U8, name="pk")
                    nc.scalar.activation(
                        s, ms[g], mybir.ActivationFunctionType.Sign, bias=biases[t]
                    )
                    out_dma(o_raw[t - N_LO], g, s)
                else:  # t == 7: fused step+spike, no membrane materialized
                    s = op_.tile([C, FW], U8, name="pk")
                    nc.vector._custom_dve(
                        LIF_SPIKE, out=s, in0=xs[(t, g)], in1=ms[g],
                        s0=th, s1=th / 2.0,
                    )
                    out_dma(o_raw[t - N_LO], g, s)


def build_nc():
    """Build + compile the per-core Bass program (cached)."""
    if "nc" in _NC_CACHE:
        return _NC_CACHE["nc"]
    nc = bacc.Bacc(
        "TRN2",
        target_bir_lowering=False,
        debug=False,
        enable_asserts=False,
        num_devices=N_CORES,
    )
    x_d = nc.dram_tensor("x", [B_LOC, T, C, H, W], FP32, kind="ExternalInput").ap()
    w_d = nc.dram_tensor("w", [C, N_LO * C], FP8, kind="ExternalInput").ap()
    o_lo = nc.dram_tensor("out_lo", [B_LOC, C, H, W], U8, kind="ExternalOutput").ap()
    o_raw = [
        nc.dram_tensor(f"out_t{t}", [B_LOC, C, H, W], U8, kind="ExternalOutput").ap()
        for t in range(N_LO, T)
    ]
    with tile.TileContext(nc) as tc:
        _emit(tc, x_d, w_d, o_lo, o_raw)
    nc.compile()
    _NC_CACHE["nc"] = nc
    return nc


def _pack_weights() -> np.ndarray:
    """[C, N_LO*C] fp8: w[c, j*C + k] = 4^j if k == c else 0."""
    np_fp8 = mybir.dt.np(FP8)
    w = np.zeros((C, N_LO, C), dtype=np_fp8)
    for j in range(N_LO):
        np.fill_diagonal(w[:, j, :], np_fp8(4.0**j))
    return np.ascontiguousarray(w.reshape(C, N_LO * C))


def make_in_maps(x: np.ndarray) -> list[dict[str, np.ndarray]]:
    assert x.shape == (B, T, C, H, W) and x.dtype == np.float32, (x.shape, x.dtype)
    w = _pack_weights()
    return [
        {"x": np.ascontiguousarray(x[i * B_LOC : (i + 1) * B_LOC]), "w": w}
        for i in range(N_CORES)
    ]


def kernel(x: np.ndarray) -> np.ndarray:
    x = np.asarray(x, dtype=np.float32)
    nc = build_nc()
    res = run_bass_kernel_spmd(nc, make_in_maps(x), list(range(N_CORES)))
    lo = np.concatenate([r["out_lo"] for r in res.results], axis=0)

    out = np.empty((B, T, C, H, W), dtype=np.float32)
    # balanced-quaternary decode of plane_lo: highest digit first
    d = lo.astype(np.int16) - int(BIAS_LO)
    for t in range(N_LO - 1, -1, -1):
        wt = 4**t
        mt = (wt - 1) // 3  # max |sum of remaining lower digits|
        st = (d > mt).astype(np.int16) - (d < -mt).astype(np.int16)
        d -= st * wt
        out[:, t] = st == 1
    for t in range(N_LO, T):
        out[:, t] = np.concatenate([r[f"out_t{t}"] for r in res.results], axis=0)
    return out
